# revision 26
# baseline (speedup 1.0000x reference)
"""Trainium2 Bass kernel for BoundaryLoss (data-parallel over batch).

Math (per batch sample b):
  mask  = boundary mask of target = (maxpool5x5(t) != minpool5x5(t)) with
          cv2-style clipped windows (OOB ignored).  Equals the reference's
          per-class dilate/erode union because a 5x5 window is non-uniform
          iff some class boundary passes through it.
  ce    = logsumexp_c(pred) - pred[t]
  wsum  = sum(mask * ce);  msum = sum(mask)
  per_sample = msum > 0 ? wsum/max(msum,1) : wsum/(H*W);  out = mean_b

Device algorithm (one sample per core):
  - pred streams in "layout B" [128, (4 rows, 512)] (partition p = rows
    4p..4p+3) giving 8KB-contiguous DMA runs (~400+ GB/s measured) — the
    21 MB pred stream is the roofline for this kernel.
  - S = sum_c exp(pred_c): exp on ACT (fp16 out), summed over classes by
    identity-matmul PSUM accumulation on TensorE.
  - picked = pred[t], mask-weighted, is gathered two ways:
      early classes (before the mask is ready): eq=(t==c) on DVE 4x, then
        copy_predicated G[t==c] = e_c into SBUF; finals add
        sum(mask*ln(G)) (G init to 1 so untouched pixels contribute 0).
      late classes (K0 < C only; currently disabled, K0=C — holding raw
        pred tiles for the mask-gated fused-stt path stalled the DMA
        stream more than the DVE savings were worth).
  - boundary mask concurrently in "layout A" [128, (4, 512)] (partition =
    row g*128+p): horizontal 5-max/min via 3 shifted tensor_tensor ops,
    PE-transpose 128x128 blocks (PSUM), vertical pools in transposed
    space, compare, PE-transpose back, bounce through DRAM into layout B.
    Emission is interleaved between class chunks so every engine's
    (statically ordered) instruction stream stays dependency-ready.
  - finals: sum(mask*ln(S)) and sum(mask*ln(G)) via stt accum; msum via
    ACT accum; partition-reduce via ones-matmuls; DMA out [1,32].
Host combines the per-core outputs.
"""

import numpy as np

B = 8
C = 21
H = 512
W = 512
N_CORES = 8
CHUNK = 2  # pred planes per DMA
K0 = 21  # classes [0, K0) use copy_predicated; [K0, C) use masked stt accum
PW = 520  # padded width of pooling buffers; data cols [2, 514)
G4 = 4  # row groups (H = G4 * 128)

_CACHE = {}


def _build_nc():
    from contextlib import ExitStack

    import concourse.bacc as bacc
    import concourse.tile as tile
    from concourse import mybir
    from concourse.masks import make_identity

    dt = mybir.dt
    Alu = mybir.AluOpType
    Act = mybir.ActivationFunctionType

    nc = bacc.Bacc("TRN2", target_bir_lowering=False, debug=False,
                   num_devices=N_CORES)

    pred = nc.dram_tensor("pred", [C, H, W], dt.float32, kind="ExternalInput")
    target = nc.dram_tensor("target", [H, W], dt.int32, kind="ExternalInput")
    out = nc.dram_tensor("out", [1, 32], dt.float32, kind="ExternalOutput")

    with tile.TileContext(nc) as tc, ExitStack() as ctx:
        consts = ctx.enter_context(tc.tile_pool(name="consts", bufs=1))
        keep = ctx.enter_context(tc.tile_pool(name="keep", bufs=1))
        mp = ctx.enter_context(tc.tile_pool(name="maskpool", bufs=1))
        ms = ctx.enter_context(tc.tile_pool(name="maskscratch", bufs=1))
        ppool = ctx.enter_context(tc.tile_pool(name="pp", bufs=3))
        epool = ctx.enter_context(tc.tile_pool(name="ep", bufs=3))
        qpool = ctx.enter_context(tc.tile_pool(name="qp", bufs=4))
        jpool = ctx.enter_context(tc.tile_pool(name="jp", bufs=2))
        opool = ctx.enter_context(tc.tile_pool(name="op", bufs=4))
        fin = ctx.enter_context(tc.tile_pool(name="fin", bufs=1))
        dramp = ctx.enter_context(tc.tile_pool(name="dram", bufs=1,
                                               space="DRAM"))
        mps = ctx.enter_context(tc.tile_pool(name="mpsum", bufs=1,
                                             space="PSUM"))
        sgp = ctx.enter_context(tc.tile_pool(name="sgpsum", bufs=1,
                                             space="PSUM"))

        ident = consts.tile([128, 128], dt.float16)
        make_identity(nc, ident)
        ones = consts.tile([128, 1], dt.float32)
        nc.gpsimd.memset(ones, 1.0)
        warm = consts.tile([128, 512], dt.float16)
        nc.gpsimd.memset(warm, 0.0)
        st_w1 = consts.tile([128, 1], dt.float32)
        st_l2 = consts.tile([128, 1], dt.float32)
        st_m = consts.tile([128, 1], dt.float32)
        st2 = consts.tile([128, 16], dt.float32)  # per-bank l2 accums
        nc.vector.memset(st2, 0.0)
        st3 = consts.tile([128, 4], dt.float32)   # per-bank w1 accums

        # layout-B tensors
        tb = keep.tile([128, G4, W], dt.float16)      # target as fp16
        maskb = keep.tile([128, G4, W], dt.float16)   # mask (from bounce)
        g_sb = keep.tile([128, 1, W], dt.float16)     # r=3 gather | 1.0
        mask_dram = dramp.tile([H, W], dt.float16)

        # ---------------- early loads ----------------
        t32 = mp.tile([128, G4, W], dt.int32)
        nc.sync.dma_start(
            out=t32, in_=target.ap().rearrange("(g p) w -> p g w", p=128))
        t32b = mp.tile([128, G4, W], dt.int32, tag="t32b")
        nc.sync.dma_start(
            out=t32b, in_=target.ap().rearrange("(p r) w -> p r w", p=128))
        nc.vector.tensor_copy(out=tb, in_=t32b)
        nc.gpsimd.memset(g_sb, 1.0)

        # PE warmup into the future S bank (discarded by c==0's start=True)
        s_ps = sgp.tile([128, G4, W], dt.float32, tag="s")
        g_ps = sgp.tile([128, 3, W], dt.float32, tag="g")
        for _ in range(10):
            nc.tensor.matmul(s_ps[:, 0, :], ident, warm, start=True,
                             stop=True)

        # ---------------- mask pipeline stages (layout A) ----------------
        xmax = mp.tile([128, G4, PW], dt.float16, tag="xmax")
        xmin = mp.tile([128, G4, PW], dt.float16, tag="xmin")
        xt = mp.tile([128, G4, PW], dt.float16, tag="xt")
        xnt = mp.tile([128, G4, PW], dt.float16, tag="xnt")
        for t in (xmax, xt):
            nc.gpsimd.memset(t[:, :, 0:2], -1.0)
            nc.gpsimd.memset(t[:, :, 2 + W:PW], -1.0)
        for t in (xmin, xnt):
            nc.gpsimd.memset(t[:, :, 0:2], 99.0)
            nc.gpsimd.memset(t[:, :, 2 + W:PW], 99.0)
        hx = mp.tile([128, G4, W], dt.float16, tag="hx")
        hn = mp.tile([128, G4, W], dt.float16, tag="hn")
        vx = mp.tile([128, G4, W], dt.float16, tag="hx")   # reuse slot
        vn = mp.tile([128, G4, W], dt.float16, tag="hn")   # reuse slot
        maskt = mp.tile([128, G4, W], dt.float16, tag="maskt")
        mask_a = mp.tile([128, G4, W], dt.float16, tag="maska")

        def pool5(src, op, dst):
            m2 = ms.tile([128, G4, PW], dt.float16, tag="m2")
            m4 = ms.tile([128, G4, PW], dt.float16, tag="m4")
            nc.vector.tensor_tensor(
                out=m2[:, :, 0:PW - 1],
                in0=src[:, :, 0:PW - 1], in1=src[:, :, 1:PW], op=op)
            nc.vector.tensor_tensor(
                out=m4[:, :, 0:PW - 3],
                in0=m2[:, :, 0:PW - 3], in1=m2[:, :, 2:PW - 1], op=op)
            nc.vector.tensor_tensor(
                out=dst, in0=m4[:, :, 0:W], in1=src[:, :, 4:4 + W], op=op)

        def tpose_in(src, dst):
            for q in range(4):
                tq = mps.tile([128, 512], dt.float16, tag="tq")
                for g in range(4):
                    nc.tensor.transpose(
                        tq[:, g * 128:(g + 1) * 128],
                        src[:, g, q * 128:(q + 1) * 128], ident)
                nc.scalar.copy(out=dst[:, q, 2:2 + W], in_=tq)

        def st_casts():
            nc.vector.tensor_copy(out=xmax[:, :, 2:2 + W], in_=t32)
            nc.vector.tensor_copy(out=xmin[:, :, 2:2 + W], in_=t32)

        def st_neq():
            nc.vector.tensor_tensor(out=maskt, in0=vx, in1=vn,
                                    op=Alu.not_equal)
            junk_m = ms.tile([128, G4, W], dt.float16, tag="junkm")
            nc.scalar.activation(out=junk_m, in_=maskt, func=Act.Copy,
                                 accum_out=st_m)

        def st_back():
            for g in range(4):
                tg = mps.tile([128, 512], dt.float16, tag="tq")
                for q in range(4):
                    nc.tensor.transpose(
                        tg[:, q * 128:(q + 1) * 128],
                        maskt[:, q, g * 128:(g + 1) * 128], ident)
                nc.scalar.copy(out=mask_a[:, g, :], in_=tg)

        def st_bounce():
            nc.gpsimd.dma_start(
                out=mask_dram[:].rearrange("(g p) w -> p g w", p=128),
                in_=mask_a)
            nc.gpsimd.dma_start(
                out=maskb,
                in_=mask_dram[:].rearrange("(p r) w -> p r w", p=128))

        def st_tt2():
            # tt2b = (t+1) * mask, in layout B
            nc.vector.scalar_tensor_tensor(
                out=tt2b, in0=tb, scalar=1.0, in1=maskb,
                op0=Alu.add, op1=Alu.mult)

        stages = [
            st_casts,
            lambda: pool5(xmax, Alu.max, hx),
            lambda: pool5(xmin, Alu.min, hn),
            lambda: tpose_in(hx, xt),
            lambda: tpose_in(hn, xnt),
            lambda: pool5(xt, Alu.max, vx),
            lambda: pool5(xnt, Alu.min, vn),
            st_neq,
            st_back,
            st_bounce,
        ]

        # ---------------- class loop (layout B), stages interleaved -------
        starts = list(range(0, C, CHUNK))
        for k, c0 in enumerate(starts):
            if k < len(stages):
                stages[k]()
            nct = min(CHUNK, C - c0)
            p_t = ppool.tile([128, nct, G4, W], dt.float32, tag="p")
            nc.sync.dma_start(
                out=p_t,
                in_=pred.ap()[c0:c0 + nct].rearrange(
                    "c (p r) w -> p c r w", p=128))
            e_t = epool.tile([128, nct, G4, W], dt.float16, tag="e")
            nc.scalar.activation(out=e_t, in_=p_t, func=Act.Exp)
            for i in range(nct):
                c = c0 + i
                eq_t = qpool.tile([128, G4, W], dt.uint16, tag="q")
                nc.vector.tensor_scalar(
                    out=eq_t, in0=tb, scalar1=float(c), scalar2=None,
                    op0=Alu.is_equal)
                # rows 0..2: gather via 2x multiply + identity matmul
                o_t = opool.tile([128, 3, W], dt.float16, tag="o")
                nc.vector.tensor_tensor(
                    out=o_t, in0=eq_t[:, 0:3, :], in1=e_t[:, i, 0:3, :],
                    op=Alu.mult)
                # row 3: gather via predicated overwrite (1x but quarter-FD)
                nc.vector.copy_predicated(out=g_sb[:, 0, :],
                                          mask=eq_t[:, 3, :],
                                          data=e_t[:, i, 3, :])
                for j in range(4):
                    nc.tensor.matmul(
                        s_ps[:, j, :], ident, e_t[:, i, j, :],
                        start=(c == 0), stop=(c == C - 1))
                for j in range(3):
                    nc.tensor.matmul(
                        g_ps[:, j, :], ident, o_t[:, j, :],
                        start=(c == 0), stop=(c == C - 1))
        for k in range(len(starts), len(stages)):
            stages[k]()

        # ---------------- finals ----------------
        # per-bank pipelined finals: each bank's ln + masked reduce starts
        # as soon as that bank's accumulation group stops
        l1 = fin.tile([128, G4, W], dt.float32)
        lg3 = fin.tile([128, 3, W], dt.float32)
        lg4 = fin.tile([128, 1, W], dt.float32)
        j1 = jpool.tile([128, G4, W], dt.float32, tag="junk")
        j2 = jpool.tile([128, G4, W], dt.float32, tag="junk")
        for j in range(4):
            nc.scalar.activation(out=l1[:, j, :], in_=s_ps[:, j, :],
                                 func=Act.Ln)
            nc.vector.scalar_tensor_tensor(
                out=j1[:, j, :], in0=l1[:, j, :], scalar=0.0,
                in1=maskb[:, j, :], op0=Alu.add, op1=Alu.mult,
                accum_out=st3[:, j:j + 1])
        for j in range(3):
            nc.scalar.activation(out=lg3[:, j, :], in_=g_ps[:, j, :],
                                 func=Act.Ln)
            nc.vector.scalar_tensor_tensor(
                out=j2[:, j, :], in0=lg3[:, j, :], scalar=0.0,
                in1=maskb[:, j, :], op0=Alu.add, op1=Alu.mult,
                accum_out=st2[:, j:j + 1])
        nc.scalar.activation(out=lg4, in_=g_sb, func=Act.Ln)
        nc.vector.scalar_tensor_tensor(
            out=j2[:, 3:4, :], in0=lg4, scalar=0.0, in1=maskb[:, 3:4, :],
            op0=Alu.add, op1=Alu.mult, accum_out=st_l2)

        # partition reductions — reuse the S bank (fully consumed by l1)
        red = s_ps[0:1, 0, 0:32]
        nc.tensor.matmul(red[:, 1:2], ones, st_l2, start=True, stop=True)
        nc.tensor.matmul(red[:, 2:3], ones, st_m, start=True, stop=True)
        nc.tensor.matmul(red[:, 4:8], ones, st3, start=True, stop=True)
        nc.tensor.matmul(red[:, 8:24], ones, st2, start=True, stop=True)
        outsb = consts.tile([1, 32], dt.float32)
        nc.vector.memset(outsb, 0.0)
        nc.vector.tensor_copy(out=outsb[:, 1:3], in_=red[:, 1:3])
        nc.vector.tensor_copy(out=outsb[:, 4:24], in_=red[:, 4:24])
        nc.sync.dma_start(out=out.ap(), in_=outsb)

    nc.compile()
    return nc


def get_nc():
    if "nc" not in _CACHE:
        _CACHE["nc"] = _build_nc()
    return _CACHE["nc"]


def _combine(outs):
    """outs: list of per-core [1,32] float32 -> scalar loss."""
    per_sample = []
    for o in outs:
        w1 = float(o[0, 4:8].sum())
        l2, msum = float(o[0, 1]), float(o[0, 2])
        l2 += float(o[0, 8:24].sum())  # auxiliary l2 partial sums
        wsum = w1 - l2
        if msum > 0:
            per_sample.append(wsum / max(msum, 1.0))
        else:
            per_sample.append(wsum / float(H * W))
    return np.float32(np.mean(per_sample))


def kernel(pred, target):
    from concourse.bass_utils import run_bass_kernel_spmd

    pred = np.ascontiguousarray(pred, dtype=np.float32)
    target = np.ascontiguousarray(target, dtype=np.int32)
    assert pred.shape == (B, C, H, W) and target.shape == (B, H, W)

    nc = get_nc()
    in_maps = [{"pred": pred[b], "target": target[b]} for b in range(B)]
    res = run_bass_kernel_spmd(nc, in_maps, core_ids=list(range(N_CORES)))
    outs = [res.results[b]["out"] for b in range(B)]
    return np.asarray(_combine(outs), dtype=np.float32)


# revision 27
# speedup vs baseline: 1.0466x; 1.0466x over previous
"""Trainium2 Bass kernel for BoundaryLoss (data-parallel over batch).

Math (per batch sample b):
  mask  = boundary mask of target = (maxpool5x5(t) != minpool5x5(t)) with
          cv2-style clipped windows (OOB ignored).  Equals the reference's
          per-class dilate/erode union because a 5x5 window is non-uniform
          iff some class boundary passes through it.
  ce    = logsumexp_c(pred) - pred[t]
  wsum  = sum(mask * ce);  msum = sum(mask)
  per_sample = msum > 0 ? wsum/max(msum,1) : wsum/(H*W);  out = mean_b

Device algorithm (one sample per core):
  - pred streams in "layout B" [128, (4 rows, 512)] (partition p = rows
    4p..4p+3) giving 8KB-contiguous DMA runs (~400+ GB/s measured) — the
    21 MB pred stream is the roofline for this kernel.
  - S = sum_c exp(pred_c): exp on ACT (fp16 out), summed over classes by
    identity-matmul PSUM accumulation on TensorE.
  - picked = pred[t], mask-weighted, is gathered two ways:
      early classes (before the mask is ready): eq=(t==c) on DVE 4x, then
        copy_predicated G[t==c] = e_c into SBUF; finals add
        sum(mask*ln(G)) (G init to 1 so untouched pixels contribute 0).
      late classes (K0 < C only; currently disabled, K0=C — holding raw
        pred tiles for the mask-gated fused-stt path stalled the DMA
        stream more than the DVE savings were worth).
  - boundary mask concurrently in "layout A" [128, (4, 512)] (partition =
    row g*128+p): horizontal 5-max/min via 3 shifted tensor_tensor ops,
    PE-transpose 128x128 blocks (PSUM), vertical pools in transposed
    space, compare, PE-transpose back, bounce through DRAM into layout B.
    Emission is interleaved between class chunks so every engine's
    (statically ordered) instruction stream stays dependency-ready.
  - finals: sum(mask*ln(S)) and sum(mask*ln(G)) via stt accum; msum via
    ACT accum; partition-reduce via ones-matmuls; DMA out [1,32].
Host combines the per-core outputs.
"""

import numpy as np

B = 8
C = 21
H = 512
W = 512
N_CORES = 8
CHUNK = 2  # pred planes per DMA
K0 = 21  # classes [0, K0) use copy_predicated; [K0, C) use masked stt accum
PW = 520  # padded width of pooling buffers; data cols [2, 514)
G4 = 4  # row groups (H = G4 * 128)

_CACHE = {}


def _build_nc():
    from contextlib import ExitStack

    import concourse.bacc as bacc
    import concourse.tile as tile
    from concourse import mybir
    from concourse.masks import make_identity

    dt = mybir.dt
    Alu = mybir.AluOpType
    Act = mybir.ActivationFunctionType

    nc = bacc.Bacc("TRN2", target_bir_lowering=False, debug=False,
                   num_devices=N_CORES)

    pred = nc.dram_tensor("pred", [C, H, W], dt.float32, kind="ExternalInput")
    target = nc.dram_tensor("target", [H, W], dt.int32, kind="ExternalInput")
    out = nc.dram_tensor("out", [1, 32], dt.float32, kind="ExternalOutput")

    with tile.TileContext(nc) as tc, ExitStack() as ctx:
        consts = ctx.enter_context(tc.tile_pool(name="consts", bufs=1))
        keep = ctx.enter_context(tc.tile_pool(name="keep", bufs=1))
        mp = ctx.enter_context(tc.tile_pool(name="maskpool", bufs=1))
        ms = ctx.enter_context(tc.tile_pool(name="maskscratch", bufs=1))
        ppool = ctx.enter_context(tc.tile_pool(name="pp", bufs=3))
        epool = ctx.enter_context(tc.tile_pool(name="ep", bufs=3))
        qpool = ctx.enter_context(tc.tile_pool(name="qp", bufs=4))
        jpool = ctx.enter_context(tc.tile_pool(name="jp", bufs=2))
        opool = ctx.enter_context(tc.tile_pool(name="op", bufs=4))
        fin = ctx.enter_context(tc.tile_pool(name="fin", bufs=1))
        dramp = ctx.enter_context(tc.tile_pool(name="dram", bufs=1,
                                               space="DRAM"))
        mps = ctx.enter_context(tc.tile_pool(name="mpsum", bufs=1,
                                             space="PSUM"))
        sgp = ctx.enter_context(tc.tile_pool(name="sgpsum", bufs=1,
                                             space="PSUM"))

        ident = consts.tile([128, 128], dt.float16)
        make_identity(nc, ident)
        ones = consts.tile([128, 1], dt.float32)
        nc.gpsimd.memset(ones, 1.0)
        warm = consts.tile([128, 512], dt.float16)
        nc.gpsimd.memset(warm, 0.0)
        st_w1 = consts.tile([128, 1], dt.float32)
        st_l2 = consts.tile([128, 1], dt.float32)
        st_m = consts.tile([128, 1], dt.float32)
        st2 = consts.tile([128, 16], dt.float32)  # auxiliary l2 accums
        nc.vector.memset(st2, 0.0)

        # layout-B tensors
        tb = keep.tile([128, G4, W], dt.float16)      # target as fp16
        maskb = keep.tile([128, G4, W], dt.float16)   # mask (from bounce)
        g_sb = keep.tile([128, 1, W], dt.float16)     # r=3 gather | 1.0
        mask_dram = dramp.tile([H, W], dt.float16)

        # ---------------- early loads ----------------
        t32 = mp.tile([128, G4, W], dt.int32)
        nc.sync.dma_start(
            out=t32, in_=target.ap().rearrange("(g p) w -> p g w", p=128))
        t32b = mp.tile([128, G4, W], dt.int32, tag="t32b")
        nc.sync.dma_start(
            out=t32b, in_=target.ap().rearrange("(p r) w -> p r w", p=128))
        nc.vector.tensor_copy(out=tb, in_=t32b)
        nc.gpsimd.memset(g_sb, 1.0)

        # PE warmup into the future S bank (discarded by c==0's start=True)
        s_ps = sgp.tile([128, G4, W], dt.float32, tag="s")
        g_ps = sgp.tile([128, 3, W], dt.float32, tag="g")
        for _ in range(10):
            nc.tensor.matmul(s_ps[:, 0, :], ident, warm, start=True,
                             stop=True)

        # ---------------- mask pipeline stages (layout A) ----------------
        xmax = mp.tile([128, G4, PW], dt.float16, tag="xmax")
        xmin = mp.tile([128, G4, PW], dt.float16, tag="xmin")
        xt = mp.tile([128, G4, PW], dt.float16, tag="xt")
        xnt = mp.tile([128, G4, PW], dt.float16, tag="xnt")
        for t in (xmax, xt):
            nc.gpsimd.memset(t[:, :, 0:2], -1.0)
            nc.gpsimd.memset(t[:, :, 2 + W:PW], -1.0)
        for t in (xmin, xnt):
            nc.gpsimd.memset(t[:, :, 0:2], 99.0)
            nc.gpsimd.memset(t[:, :, 2 + W:PW], 99.0)
        hx = mp.tile([128, G4, W], dt.float16, tag="hx")
        hn = mp.tile([128, G4, W], dt.float16, tag="hn")
        vx = mp.tile([128, G4, W], dt.float16, tag="hx")   # reuse slot
        vn = mp.tile([128, G4, W], dt.float16, tag="hn")   # reuse slot
        maskt = mp.tile([128, G4, W], dt.float16, tag="maskt")
        mask_a = mp.tile([128, G4, W], dt.float16, tag="maska")

        def pool5(src, op, dst):
            m2 = ms.tile([128, G4, PW], dt.float16, tag="m2")
            m4 = ms.tile([128, G4, PW], dt.float16, tag="m4")
            nc.vector.tensor_tensor(
                out=m2[:, :, 0:PW - 1],
                in0=src[:, :, 0:PW - 1], in1=src[:, :, 1:PW], op=op)
            nc.vector.tensor_tensor(
                out=m4[:, :, 0:PW - 3],
                in0=m2[:, :, 0:PW - 3], in1=m2[:, :, 2:PW - 1], op=op)
            nc.vector.tensor_tensor(
                out=dst, in0=m4[:, :, 0:W], in1=src[:, :, 4:4 + W], op=op)

        def tpose_in(src, dst):
            for q in range(4):
                tq = mps.tile([128, 512], dt.float16, tag="tq")
                for g in range(4):
                    nc.tensor.transpose(
                        tq[:, g * 128:(g + 1) * 128],
                        src[:, g, q * 128:(q + 1) * 128], ident)
                nc.scalar.copy(out=dst[:, q, 2:2 + W], in_=tq)

        def st_casts():
            nc.vector.tensor_copy(out=xmax[:, :, 2:2 + W], in_=t32)
            nc.vector.tensor_copy(out=xmin[:, :, 2:2 + W], in_=t32)

        def st_neq():
            nc.vector.tensor_tensor(out=maskt, in0=vx, in1=vn,
                                    op=Alu.not_equal)
            junk_m = ms.tile([128, G4, W], dt.float16, tag="junkm")
            nc.scalar.activation(out=junk_m, in_=maskt, func=Act.Copy,
                                 accum_out=st_m)

        def st_back():
            for g in range(4):
                tg = mps.tile([128, 512], dt.float16, tag="tq")
                for q in range(4):
                    nc.tensor.transpose(
                        tg[:, q * 128:(q + 1) * 128],
                        maskt[:, q, g * 128:(g + 1) * 128], ident)
                nc.scalar.copy(out=mask_a[:, g, :], in_=tg)

        def st_bounce():
            nc.gpsimd.dma_start(
                out=mask_dram[:].rearrange("(g p) w -> p g w", p=128),
                in_=mask_a)
            nc.gpsimd.dma_start(
                out=maskb,
                in_=mask_dram[:].rearrange("(p r) w -> p r w", p=128))

        def st_tt2():
            # tt2b = (t+1) * mask, in layout B
            nc.vector.scalar_tensor_tensor(
                out=tt2b, in0=tb, scalar=1.0, in1=maskb,
                op0=Alu.add, op1=Alu.mult)

        stages = [
            st_casts,
            lambda: pool5(xmax, Alu.max, hx),
            lambda: pool5(xmin, Alu.min, hn),
            lambda: tpose_in(hx, xt),
            lambda: tpose_in(hn, xnt),
            lambda: pool5(xt, Alu.max, vx),
            lambda: pool5(xnt, Alu.min, vn),
            st_neq,
            st_back,
            st_bounce,
        ]

        # ---------------- class loop (layout B), stages interleaved -------
        starts = list(range(0, C, CHUNK))
        for k, c0 in enumerate(starts):
            if k < len(stages):
                stages[k]()
            nct = min(CHUNK, C - c0)
            p_t = ppool.tile([128, nct, G4, W], dt.float32, tag="p")
            nc.sync.dma_start(
                out=p_t,
                in_=pred.ap()[c0:c0 + nct].rearrange(
                    "c (p r) w -> p c r w", p=128))
            e_t = epool.tile([128, nct, G4, W], dt.float16, tag="e")
            nc.scalar.activation(out=e_t, in_=p_t, func=Act.Exp)
            for i in range(nct):
                c = c0 + i
                eq_t = qpool.tile([128, G4, W], dt.uint16, tag="q")
                nc.vector.tensor_scalar(
                    out=eq_t, in0=tb, scalar1=float(c), scalar2=None,
                    op0=Alu.is_equal)
                # rows 0..2: gather via 2x multiply + identity matmul
                o_t = opool.tile([128, 3, W], dt.float16, tag="o")
                nc.vector.tensor_tensor(
                    out=o_t, in0=eq_t[:, 0:3, :], in1=e_t[:, i, 0:3, :],
                    op=Alu.mult)
                # row 3: gather via predicated overwrite (1x but quarter-FD)
                nc.vector.copy_predicated(out=g_sb[:, 0, :],
                                          mask=eq_t[:, 3, :],
                                          data=e_t[:, i, 3, :])
                for j in range(4):
                    nc.tensor.matmul(
                        s_ps[:, j, :], ident, e_t[:, i, j, :],
                        start=(c == 0), stop=(c == C - 1))
                for j in range(3):
                    nc.tensor.matmul(
                        g_ps[:, j, :], ident, o_t[:, j, :],
                        start=(c == 0), stop=(c == C - 1))
        for k in range(len(starts), len(stages)):
            stages[k]()

        # ---------------- finals ----------------
        l1 = fin.tile([128, G4, W], dt.float32)
        nc.scalar.activation(out=l1, in_=s_ps, func=Act.Ln)
        lg3 = fin.tile([128, 3, W], dt.float32)
        nc.scalar.activation(out=lg3, in_=g_ps, func=Act.Ln)
        lg4 = fin.tile([128, 1, W], dt.float32)
        nc.scalar.activation(out=lg4, in_=g_sb, func=Act.Ln)

        j1 = jpool.tile([128, G4, W], dt.float32, tag="junk")
        nc.vector.scalar_tensor_tensor(
            out=j1, in0=l1, scalar=0.0, in1=maskb,
            op0=Alu.add, op1=Alu.mult, accum_out=st_w1)
        j2 = jpool.tile([128, G4, W], dt.float32, tag="junk")
        nc.vector.scalar_tensor_tensor(
            out=j2[:, 0:3, :], in0=lg3, scalar=0.0, in1=maskb[:, 0:3, :],
            op0=Alu.add, op1=Alu.mult, accum_out=st_l2)
        nc.vector.scalar_tensor_tensor(
            out=j2[:, 3:4, :], in0=lg4, scalar=0.0, in1=maskb[:, 3:4, :],
            op0=Alu.add, op1=Alu.mult, accum_out=st2[:, 0:1])

        # partition reductions — reuse the S bank (fully consumed by l1)
        red = s_ps[0:1, 0, 0:32]
        nc.tensor.matmul(red[:, 0:1], ones, st_w1, start=True, stop=True)
        nc.tensor.matmul(red[:, 1:2], ones, st_l2, start=True, stop=True)
        nc.tensor.matmul(red[:, 2:3], ones, st_m, start=True, stop=True)
        nc.tensor.matmul(red[:, 8:24], ones, st2, start=True, stop=True)
        outsb = consts.tile([1, 32], dt.float32)
        nc.vector.memset(outsb, 0.0)
        nc.vector.tensor_copy(out=outsb[:, 0:3], in_=red[:, 0:3])
        nc.vector.tensor_copy(out=outsb[:, 8:24], in_=red[:, 8:24])
        nc.sync.dma_start(out=out.ap(), in_=outsb)

    nc.compile()
    return nc


def get_nc():
    if "nc" not in _CACHE:
        _CACHE["nc"] = _build_nc()
    return _CACHE["nc"]


def _combine(outs):
    """outs: list of per-core [1,32] float32 -> scalar loss."""
    per_sample = []
    for o in outs:
        w1, l2, msum = float(o[0, 0]), float(o[0, 1]), float(o[0, 2])
        l2 += float(o[0, 8:24].sum())  # auxiliary l2 partial sums
        wsum = w1 - l2
        if msum > 0:
            per_sample.append(wsum / max(msum, 1.0))
        else:
            per_sample.append(wsum / float(H * W))
    return np.float32(np.mean(per_sample))


def kernel(pred, target):
    from concourse.bass_utils import run_bass_kernel_spmd

    pred = np.ascontiguousarray(pred, dtype=np.float32)
    target = np.ascontiguousarray(target, dtype=np.int32)
    assert pred.shape == (B, C, H, W) and target.shape == (B, H, W)

    nc = get_nc()
    in_maps = [{"pred": pred[b], "target": target[b]} for b in range(B)]
    res = run_bass_kernel_spmd(nc, in_maps, core_ids=list(range(N_CORES)))
    outs = [res.results[b]["out"] for b in range(B)]
    return np.asarray(_combine(outs), dtype=np.float32)
